# revision 18
# baseline (speedup 1.0000x reference)
"""Trainium2 Bass kernel for nn_BitwiseLinear: y = x @ tanh(W).T

Full problem: x [32768, 8192] f32, W [256, 8192] f32 -> y [32768, 256] f32.

Data-parallel over 8 NeuronCores: core c computes
    y[c*4096:(c+1)*4096, :] = x_shard @ tanh(W).T
with W replicated (tanh computed redundantly per core on ScalarE).

Device layout choices (all prepared host-side, so every DMA is contiguous):
  x  -> fp16, shard transposed to [tc, p, blk, tl]  (tc = token chunk of 512,
        blk*128+p = contraction index i, tl = token within chunk)
  w  -> fp16, transposed to [p, blk, o]
  out <- fp16 [256, 4096] = y_shard.T  (o on partitions)

Matmul: out_psum[o_tile 128, t 512] += wT[i 128, o 128].T @ xT[i 128, t 512],
accumulated over 64 i-blocks in PSUM (fp32), fp16 operands at 1 cycle/row.
"""

import numpy as np

TOKENS = 32768
IN_DIM = 8192
OUT_DIM = 256
N_CORES = 8
TPC = TOKENS // N_CORES        # 4096 tokens per core
TCHUNK = 512                   # tokens per PSUM tile (matmul free dim)
NTC = TPC // TCHUNK            # 8 token chunks per core
P = 128
NBLK = IN_DIM // P             # 64 contraction blocks
GBLK = 16                      # blocks per x DMA group (2 MB transfers)
NGRP = NBLK // GBLK            # 4 groups
NOT = OUT_DIM // P             # 2 output-row tiles

_NC_CACHE = {}


def _build_nc():
    import concourse.mybir as mybir
    import concourse.tile as tile
    from concourse import bacc

    fp16 = mybir.dt.float16
    f32 = mybir.dt.float32

    nc = bacc.Bacc(
        "TRN2", target_bir_lowering=False, debug=False, num_devices=N_CORES
    )
    X = nc.dram_tensor("x", [NTC, P, NBLK, TCHUNK], fp16, kind="ExternalInput").ap()
    W = nc.dram_tensor("w", [P, NBLK, OUT_DIM], fp16, kind="ExternalInput").ap()
    OUT = nc.dram_tensor("out", [OUT_DIM, TPC], fp16, kind="ExternalOutput").ap()

    with tile.TileContext(nc) as tc:
        with (
            tc.tile_pool(name="wraw", bufs=2) as wraw_pool,
            tc.tile_pool(name="wtanh", bufs=NGRP) as wt_pool,
            tc.tile_pool(name="xp", bufs=7) as xpool,
            tc.tile_pool(name="ya", bufs=NOT) as yacc_pool,
            tc.tile_pool(name="yp", bufs=4) as ypool,
            tc.tile_pool(name="ps", bufs=4, space="PSUM") as pspool,
        ):
            # Weight-stationary phases: outer loop over the NGRP weight
            # groups, inner loop over all NTC token chunks. Partial sums
            # accumulate in an SBUF f32 tensor between phases. This keeps
            # the x-DMA demand per phase (~16 MB) below the PE time per
            # phase (~55 us), so W's 4 MB never starves the x stream.
            SUB = 4  # blocks per startup sub-DMA
            wrs = [
                wraw_pool.tile([P, GBLK, OUT_DIM], fp16, name=f"wr{g}", tag="wr")
                for g in range(NGRP)
            ]
            wts = [
                wt_pool.tile([P, GBLK, OUT_DIM], fp16, name=f"wa{g}", tag="wa")
                for g in range(NGRP)
            ]
            yacc = [
                yacc_pool.tile([P, TPC], fp16, name=f"yacc{o}", tag="ya")
                for o in range(NOT)
            ]

            # PE warm-up: the HAM clock gate keeps the PE at 1.2 GHz until
            # it has been busy ~3.4 us. Run throwaway matmuls on zeroed
            # scratch during the DMA-start dead window so the real stream
            # begins at 2.4 GHz.
            scr = ypool.tile([P, P], fp16, name="warm_scr", tag="warm_scr")
            scr_ps = pspool.tile([P, P], f32, name="warm_ps", tag="warm_ps")
            nc.vector.memset(scr[:], 0.0)
            for _ in range(40):
                nc.tensor.matmul(
                    scr_ps[:, :], lhsT=scr[:, :], rhs=scr[:, :],
                    start=True, stop=True,
                )

            # Startup-critical: first matmul needs tanh(W[blk 0..1]) and
            # x[tc0, blk 0..1] only — issue those as small interleaved
            # sub-DMAs so the PE starts within a few microseconds.
            xtiles = {}
            xtiles[(0, 0)] = xpool.tile(
                [P, GBLK, TCHUNK], fp16, name="xt0_0", tag="xt"
            )
            # w sub-loads go on the SP HWDGE ring, x sub-loads on the ACT
            # ring — the two rings drain in parallel into the same 16 SDMA
            # engines, so the startup-critical data lands sooner.
            subs = [(0, 2), (2, 2), (4, 4), (8, 4), (12, 4)]
            for j, n in subs:
                nc.sync.dma_start(
                    out=wrs[0][:, j : j + n, :], in_=W[:, j : j + n, :]
                )
                nc.sync.dma_start(
                    out=xtiles[(0, 0)][:, j : j + n, :],
                    in_=X[0, :, j : j + n, :],
                )
                nc.scalar.activation(
                    wts[0][:, j : j + n, :],
                    wrs[0][:, j : j + n, :],
                    mybir.ActivationFunctionType.Tanh,
                )

            def issue_x(g, t):
                xt = xpool.tile(
                    [P, GBLK, TCHUNK], fp16, name=f"xt{g}_{t}", tag="xt"
                )
                nc.sync.dma_start(
                    out=xt[:], in_=X[t, :, g * GBLK : (g + 1) * GBLK, :]
                )
                xtiles[(g, t)] = xt

            def issue_w(g):
                nc.sync.dma_start(
                    out=wrs[g][:], in_=W[:, g * GBLK : (g + 1) * GBLK, :]
                )
                nc.scalar.activation(
                    wts[g][:], wrs[g][:], mybir.ActivationFunctionType.Tanh
                )

            # Remaining x tiles for phase 0. Only w group 1 loads during
            # phase 0 — groups 2/3 aren't needed until ~55/~82 us, so
            # deferring them keeps the startup-critical window lean.
            for t in range(1, NTC):
                issue_x(0, t)
                if t < NGRP:
                    issue_w(t)

            for g in range(NGRP):
                for t in range(NTC):
                    if (g, t) not in xtiles:
                        issue_x(g, t)
                    xt = xtiles.pop((g, t))
                    psums = [
                        pspool.tile(
                            [P, TCHUNK], f32, name=f"ps_{g}_{t}_{o}", tag="ps"
                        )
                        for o in range(NOT)
                    ]
                    for bl in range(GBLK):
                        for o in range(NOT):
                            nc.tensor.matmul(
                                psums[o][:, :],
                                lhsT=wts[g][:, bl, o * P : (o + 1) * P],
                                rhs=xt[:, bl, :],
                                start=(bl == 0),
                                stop=(bl == GBLK - 1),
                            )
                    tsl = slice(t * TCHUNK, (t + 1) * TCHUNK)
                    for o in range(NOT):
                        if g == 0:
                            nc.vector.tensor_copy(
                                yacc[o][:, tsl], psums[o][:, :]
                            )
                        elif g < NGRP - 1:
                            nc.vector.tensor_add(
                                yacc[o][:, tsl], psums[o][:, :], yacc[o][:, tsl]
                            )
                        else:
                            ysb = ypool.tile(
                                [P, TCHUNK], fp16, name=f"ysb{t}_{o}", tag="ysb"
                            )
                            nc.vector.tensor_add(
                                ysb[:], psums[o][:, :], yacc[o][:, tsl]
                            )
                            # ACT HWDGE queue: don't serialize behind x loads.
                            nc.scalar.dma_start(
                                out=OUT[o * P : (o + 1) * P, tsl], in_=ysb[:]
                            )
    nc.compile()
    return nc


def _get_nc():
    if "nc" not in _NC_CACHE:
        _NC_CACHE["nc"] = _build_nc()
    return _NC_CACHE["nc"]


def _prep_inputs(x, weight):
    """Host-side shard + layout. Returns in_maps for the 8 cores."""
    w16 = np.ascontiguousarray(
        weight.T.astype(np.float16)          # [8192, 256] = [i, o]
        .reshape(NBLK, P, OUT_DIM)           # [blk, p, o]
        .transpose(1, 0, 2)                  # [p, blk, o]
    )
    in_maps = []
    for c in range(N_CORES):
        xc = x[c * TPC : (c + 1) * TPC].astype(np.float16)  # [4096, 8192]
        xl = np.ascontiguousarray(
            xc.reshape(NTC, TCHUNK, NBLK, P)  # [tc, tl, blk, p]
            .transpose(0, 3, 2, 1)            # [tc, p, blk, tl]
        )
        in_maps.append({"x": xl, "w": w16})
    return in_maps


def run(x, weight, trace=False):
    """Run on hardware; returns (y, BassKernelResults)."""
    from concourse.bass_utils import run_bass_kernel_spmd

    nc = _get_nc()
    in_maps = _prep_inputs(x, weight)
    res = run_bass_kernel_spmd(
        nc, in_maps, core_ids=list(range(N_CORES)), trace=trace
    )
    y = np.concatenate(
        [res.results[c]["out"].astype(np.float32).T for c in range(N_CORES)],
        axis=0,
    )
    return y, res


def kernel(x, weight):
    y, _ = run(np.asarray(x), np.asarray(weight), trace=False)
    return y


# revision 19
# speedup vs baseline: 1.0223x; 1.0223x over previous
"""Trainium2 Bass kernel for nn_BitwiseLinear: y = x @ tanh(W).T

Full problem: x [32768, 8192] f32, W [256, 8192] f32 -> y [32768, 256] f32.

Data-parallel over 8 NeuronCores: core c computes
    y[c*4096:(c+1)*4096, :] = x_shard @ tanh(W).T
with W replicated (tanh computed redundantly per core on ScalarE).

Device layout choices (all prepared host-side, so every DMA is contiguous):
  x  -> fp16, shard transposed to [tc, p, blk, tl]  (tc = token chunk of 512,
        blk*128+p = contraction index i, tl = token within chunk)
  w  -> fp16, transposed to [p, blk, o]
  out <- fp16 [256, 4096] = y_shard.T  (o on partitions)

Matmul: out_psum[o_tile 128, t 512] += wT[i 128, o 128].T @ xT[i 128, t 512],
accumulated over 64 i-blocks in PSUM (fp32), fp16 operands at 1 cycle/row.
"""

import numpy as np

TOKENS = 32768
IN_DIM = 8192
OUT_DIM = 256
N_CORES = 8
TPC = TOKENS // N_CORES        # 4096 tokens per core
TCHUNK = 512                   # tokens per PSUM tile (matmul free dim)
NTC = TPC // TCHUNK            # 8 token chunks per core
P = 128
NBLK = IN_DIM // P             # 64 contraction blocks
GBLK = 16                      # blocks per x DMA group (2 MB transfers)
NGRP = NBLK // GBLK            # 4 groups
NOT = OUT_DIM // P             # 2 output-row tiles

_NC_CACHE = {}


def _build_nc():
    import concourse.mybir as mybir
    import concourse.tile as tile
    from concourse import bacc

    fp16 = mybir.dt.float16
    f32 = mybir.dt.float32

    nc = bacc.Bacc(
        "TRN2", target_bir_lowering=False, debug=False, num_devices=N_CORES
    )
    X = nc.dram_tensor("x", [NTC, P, NBLK, TCHUNK], fp16, kind="ExternalInput").ap()
    W = nc.dram_tensor("w", [P, NBLK, OUT_DIM], fp16, kind="ExternalInput").ap()
    OUT = nc.dram_tensor("out", [OUT_DIM, TPC], fp16, kind="ExternalOutput").ap()

    with tile.TileContext(nc) as tc:
        with (
            tc.tile_pool(name="wraw", bufs=2) as wraw_pool,
            tc.tile_pool(name="wtanh", bufs=NGRP) as wt_pool,
            tc.tile_pool(name="xp", bufs=6) as xpool,
            tc.tile_pool(name="ya", bufs=NOT) as yacc_pool,
            tc.tile_pool(name="yp", bufs=4) as ypool,
            tc.tile_pool(name="ps", bufs=4, space="PSUM") as pspool,
        ):
            # Weight-stationary phases: outer loop over the NGRP weight
            # groups, inner loop over all NTC token chunks. Partial sums
            # accumulate in an SBUF f32 tensor between phases. This keeps
            # the x-DMA demand per phase (~16 MB) below the PE time per
            # phase (~55 us), so W's 4 MB never starves the x stream.
            SUB = 4  # blocks per startup sub-DMA
            wrs = [
                wraw_pool.tile([P, GBLK, OUT_DIM], fp16, name=f"wr{g}", tag="wr")
                for g in range(NGRP)
            ]
            wts = [
                wt_pool.tile([P, GBLK, OUT_DIM], fp16, name=f"wa{g}", tag="wa")
                for g in range(NGRP)
            ]
            yacc = [
                yacc_pool.tile([P, TPC], fp16, name=f"yacc{o}", tag="ya")
                for o in range(NOT)
            ]

            # PE warm-up: the HAM clock gate keeps the PE at 1.2 GHz until
            # it has been busy ~3.4 us. Run throwaway matmuls on zeroed
            # scratch during the DMA-start dead window so the real stream
            # begins at 2.4 GHz.
            scr = ypool.tile([P, P], fp16, name="warm_scr", tag="warm_scr")
            scr_ps = pspool.tile([P, P], f32, name="warm_ps", tag="warm_ps")
            nc.vector.memset(scr[:], 0.0)
            for _ in range(40):
                nc.tensor.matmul(
                    scr_ps[:, :], lhsT=scr[:, :], rhs=scr[:, :],
                    start=True, stop=True,
                )

            # Startup-critical: first matmul needs tanh(W[blk 0..1]) and
            # x[tc0, blk 0..1] only — issue those as small interleaved
            # sub-DMAs so the PE starts within a few microseconds.
            xtiles = {}
            xtiles[(0, 0)] = xpool.tile(
                [P, GBLK, TCHUNK], fp16, name="xt0_0", tag="xt"
            )
            # w sub-loads go on the SP HWDGE ring, x sub-loads on the ACT
            # ring — the two rings drain in parallel into the same 16 SDMA
            # engines, so the startup-critical data lands sooner.
            subs = [(0, 2), (2, 2), (4, 4), (8, 4), (12, 4)]
            for j, n in subs:
                nc.sync.dma_start(
                    out=wrs[0][:, j : j + n, :], in_=W[:, j : j + n, :]
                )
                nc.sync.dma_start(
                    out=xtiles[(0, 0)][:, j : j + n, :],
                    in_=X[0, :, j : j + n, :],
                )
                nc.scalar.activation(
                    wts[0][:, j : j + n, :],
                    wrs[0][:, j : j + n, :],
                    mybir.ActivationFunctionType.Tanh,
                )

            def issue_x(g, t):
                xt = xpool.tile(
                    [P, GBLK, TCHUNK], fp16, name=f"xt{g}_{t}", tag="xt"
                )
                nc.sync.dma_start(
                    out=xt[:], in_=X[t, :, g * GBLK : (g + 1) * GBLK, :]
                )
                xtiles[(g, t)] = xt

            def issue_w(g):
                nc.sync.dma_start(
                    out=wrs[g][:], in_=W[:, g * GBLK : (g + 1) * GBLK, :]
                )
                nc.scalar.activation(
                    wts[g][:], wrs[g][:], mybir.ActivationFunctionType.Tanh
                )

            # Remaining x tiles for phase 0. Only w group 1 loads during
            # phase 0 — groups 2/3 aren't needed until ~55/~82 us, so
            # deferring them keeps the startup-critical window lean.
            # W groups 1-3 are first needed at ~67/~121/~176 us; issue
            # them behind x(0,4..6) so the startup-critical x stream is
            # never displaced by weight bytes.
            for t in range(1, NTC):
                issue_x(0, t)
                if 4 <= t <= 6:
                    issue_w(t - 3)

            for g in range(NGRP):
                for t in range(NTC):
                    if (g, t) not in xtiles:
                        issue_x(g, t)
                    xt = xtiles.pop((g, t))
                    psums = [
                        pspool.tile(
                            [P, TCHUNK], f32, name=f"ps_{g}_{t}_{o}", tag="ps"
                        )
                        for o in range(NOT)
                    ]
                    for bl in range(GBLK):
                        for o in range(NOT):
                            nc.tensor.matmul(
                                psums[o][:, :],
                                lhsT=wts[g][:, bl, o * P : (o + 1) * P],
                                rhs=xt[:, bl, :],
                                start=(bl == 0),
                                stop=(bl == GBLK - 1),
                            )
                    tsl = slice(t * TCHUNK, (t + 1) * TCHUNK)
                    for o in range(NOT):
                        if g == 0:
                            nc.vector.tensor_copy(
                                yacc[o][:, tsl], psums[o][:, :]
                            )
                        elif g < NGRP - 1:
                            nc.vector.tensor_add(
                                yacc[o][:, tsl], psums[o][:, :], yacc[o][:, tsl]
                            )
                        else:
                            ysb = ypool.tile(
                                [P, TCHUNK], fp16, name=f"ysb{t}_{o}", tag="ysb"
                            )
                            nc.vector.tensor_add(
                                ysb[:], psums[o][:, :], yacc[o][:, tsl]
                            )
                            # ACT HWDGE queue: don't serialize behind x loads.
                            nc.scalar.dma_start(
                                out=OUT[o * P : (o + 1) * P, tsl], in_=ysb[:]
                            )
    nc.compile()
    return nc


def _get_nc():
    if "nc" not in _NC_CACHE:
        _NC_CACHE["nc"] = _build_nc()
    return _NC_CACHE["nc"]


def _prep_inputs(x, weight):
    """Host-side shard + layout. Returns in_maps for the 8 cores."""
    w16 = np.ascontiguousarray(
        weight.T.astype(np.float16)          # [8192, 256] = [i, o]
        .reshape(NBLK, P, OUT_DIM)           # [blk, p, o]
        .transpose(1, 0, 2)                  # [p, blk, o]
    )
    in_maps = []
    for c in range(N_CORES):
        xc = x[c * TPC : (c + 1) * TPC].astype(np.float16)  # [4096, 8192]
        xl = np.ascontiguousarray(
            xc.reshape(NTC, TCHUNK, NBLK, P)  # [tc, tl, blk, p]
            .transpose(0, 3, 2, 1)            # [tc, p, blk, tl]
        )
        in_maps.append({"x": xl, "w": w16})
    return in_maps


def run(x, weight, trace=False):
    """Run on hardware; returns (y, BassKernelResults)."""
    from concourse.bass_utils import run_bass_kernel_spmd

    nc = _get_nc()
    in_maps = _prep_inputs(x, weight)
    res = run_bass_kernel_spmd(
        nc, in_maps, core_ids=list(range(N_CORES)), trace=trace
    )
    y = np.concatenate(
        [res.results[c]["out"].astype(np.float32).T for c in range(N_CORES)],
        axis=0,
    )
    return y, res


def kernel(x, weight):
    y, _ = run(np.asarray(x), np.asarray(weight), trace=False)
    return y
